# revision 19
# baseline (speedup 1.0000x reference)
"""Bass/Trainium2 kernel for nn_CGRE_68719477510 (ragged_sequence).

Restructure: scores[i] = X[i] . Constraints[rel(bag(i))] and the classifier
out = bag @ W.T are both projections of X onto small [53, 2070] matrices.
So one device pass computes Y = [Constraints; W] @ X.T  ([106, N]) — the only
traffic proportional to X (543 MB). The segment softmax + weighted sum then
operate on the projected [N, 53] rows (P = X @ W.T), never touching X again:
    out[bag] = sum_i softmax_i(S) * P[i]  ==  (sum_i w_i X_i) @ W.T
Sharding: split sentences N=65536 into 8 contiguous chunks of 8192 (one per
core); replicate the small combined weight. The ragged segment ops run on
host over the tiny [N, 53] projection.

Device matmul precision: X is shipped as a bf16 hi/lo pair (same bytes as
f32) and Y is accumulated from three exact-product bf16 matmuls
(Xh.[Ch|Wh] + Xh.[Cl|Wl] + Xl.[Ch|Wh]); only the ~2^-18 Xl.Cl term is
dropped, giving near-fp32 scores at full PE streaming rate.
"""

import sys

sys.path.insert(0, "/opt/trn_rl_repo")

import numpy as np

N_SENT = 65536
D_FEAT = 2070
N_REL = 53
N_CORES = 8
N_PER_CORE = N_SENT // N_CORES  # 8192
M_OUT = 2 * N_REL  # 106 rows: [Constraints; W]

KC = 128                      # contraction chunk (partition dim)
N_SUPER = 4096                # sentences per supergroup (fills all 8 PSUM banks)
MM_N = 512                    # moving free dim per matmul (one PSUM bank)
N_KCHUNKS = (D_FEAT + KC - 1) // KC  # 17 (16x128 + 22)

VARIANT = "fp16"              # "fp16" | "bf16split" | "f32r"

_CACHE = {}


def _build_fp16():
    """Single-pass fp16 variant: X and [Constraints; W] both fp16.

    Halves HBM traffic vs the bf16 hi/lo pair (33.9 MB/core vs 67.9) and
    cuts PE work 3x (one matmul per tile). fp16's 10-bit mantissa keeps
    score error ~0.01 abs -> ~0.14% output Frobenius error (measured on
    the real inputs), 15x under the 2e-2 gate.
    """
    import concourse.mybir as mybir
    from concourse import bacc
    from concourse.tile import TileContext

    F16 = mybir.dt.float16
    F32 = mybir.dt.float32
    I16 = mybir.dt.int16
    MP = 128  # stationary padded to 128: C rows at 0..52, W rows at 64..116
              # (PSUM partition reads must start quadrant-aligned: 0 / 64)

    nc = bacc.Bacc("TRN2", target_bir_lowering=False, debug=True)
    xt = nc.dram_tensor("xt", [D_FEAT, N_PER_CORE], F16, kind="ExternalInput")
    cw = nc.dram_tensor("cw", [KC, N_KCHUNKS * MP], F16, kind="ExternalInput")

    n_super = 2048                    # 4 PSUM banks -> ping-pong 2 tiles
    n_supers = N_PER_CORE // n_super  # 4
    subs = n_super // MM_N            # 4

    # DMA writes pin to one engine per queue (sync->0, scalar->1, gpsimd->
    # {6,7}) instead of spraying like strided reads, so output bytes are the
    # scarce resource: scores ship as int16 (x64 fixed point, |s|<240 so no
    # overflow) and P as fp16, balanced across all four write sinks.
    ys = nc.dram_tensor("ys", [n_supers, N_REL, n_super], I16, kind="ExternalOutput")
    yp = nc.dram_tensor("yp", [n_supers, N_REL, n_super], F16, kind="ExternalOutput")

    with TileContext(nc) as tc:
        with (
            tc.tile_pool(name="w", bufs=1) as wpool,
            tc.tile_pool(name="x", bufs=20) as xpool,
            tc.tile_pool(name="out", bufs=4) as opool,
            tc.tile_pool(name="psum", bufs=2, space="PSUM") as ppool,
        ):
            wtile = wpool.tile([KC, N_KCHUNKS * MP], F16, tag="w")
            nc.sync.dma_start(out=wtile[:, :], in_=cw[:, :])

            # ALL output writes go through gpsimd's software DGE queues.
            # x loads and hwdge writes share the 8 DMAHW hardware queues and
            # their +16-per-transfer semaphore accounting; single-engine-
            # pinned writes break the expected increment pattern and stall
            # the load stream ~15us at every supergroup boundary. The swdge
            # (DMASW) queues are accounted separately, so writes there never
            # perturb the load pipeline (engines 6/7 absorb ~17us of write
            # descriptors, still below the 83us x floor).
            for sp in range(n_supers):
                last = sp == n_supers - 1
                c0 = sp * n_super
                psum = ppool.tile([MP, n_super], F32, tag="ps")
                ys_t = opool.tile([N_REL, n_super], I16, tag="ys")
                yp_t = opool.tile([N_REL, n_super], F16, tag="yp")
                if not last:
                    for k in range(N_KCHUNKS):
                        k0 = k * KC
                        kp = min(KC, D_FEAT - k0)
                        xtile = xpool.tile([KC, n_super], F16, tag="x")
                        eng = nc.sync if k % 2 == 0 else nc.scalar
                        eng.dma_start(
                            out=xtile[:kp], in_=xt[k0 : k0 + kp, c0 : c0 + n_super]
                        )
                        ws = slice(k * MP, (k + 1) * MP)
                        for s in range(subs):
                            nc.tensor.matmul(
                                psum[:, s * MM_N : (s + 1) * MM_N],
                                wtile[:kp, ws],
                                xtile[:kp, s * MM_N : (s + 1) * MM_N],
                                start=(k == 0),
                                stop=(k == N_KCHUNKS - 1),
                            )
                    nc.vector.tensor_scalar_mul(ys_t[:, :], psum[:N_REL], 64.0)
                    nc.vector.tensor_copy(
                        out=yp_t[:, :], in_=psum[64 : 64 + N_REL]
                    )
                    nc.gpsimd.dma_start(out=ys[sp], in_=ys_t[:, :])
                    nc.gpsimd.dma_start(out=yp[sp], in_=yp_t[:, :])
                else:
                    # last super runs column-major (s outer): each 512-col
                    # chunk finishes all k-steps early, so its cast + write
                    # pipeline behind the remaining matmuls instead of
                    # forming a naked tail after the final matmul
                    xts = []
                    for k in range(N_KCHUNKS):
                        k0 = k * KC
                        kp = min(KC, D_FEAT - k0)
                        xtile = xpool.tile([KC, n_super], F16, tag="x")
                        eng = nc.sync if k % 2 == 0 else nc.scalar
                        eng.dma_start(
                            out=xtile[:kp], in_=xt[k0 : k0 + kp, c0 : c0 + n_super]
                        )
                        xts.append((xtile, kp))
                    for s in range(subs):
                        cs = slice(s * MM_N, (s + 1) * MM_N)
                        for k in range(N_KCHUNKS):
                            xtile, kp = xts[k]
                            nc.tensor.matmul(
                                psum[:, cs],
                                wtile[:kp, k * MP : (k + 1) * MP],
                                xtile[:kp, cs],
                                start=(k == 0),
                                stop=(k == N_KCHUNKS - 1),
                            )
                        nc.vector.tensor_scalar_mul(
                            ys_t[:, cs], psum[:N_REL, cs], 64.0
                        )
                        nc.vector.tensor_copy(
                            out=yp_t[:, cs], in_=psum[64 : 64 + N_REL, cs]
                        )
                        nc.gpsimd.dma_start(out=ys[sp, :, cs], in_=ys_t[:, cs])
                        nc.gpsimd.dma_start(out=yp[sp, :, cs], in_=yp_t[:, cs])

    nc.compile()
    return nc, n_super


def _build_f32r():
    import concourse.mybir as mybir
    from concourse import bacc
    from concourse.tile import TileContext

    DT = mybir.dt.float32r  # fp32 bits, full-rate PE streaming mode
    F32 = mybir.dt.float32

    nc = bacc.Bacc("TRN2", target_bir_lowering=False, debug=True)
    xt = nc.dram_tensor("xt", [D_FEAT, N_PER_CORE], DT, kind="ExternalInput")
    # weights packed on host: wpack[p, k*106+m] = CW[m, 128k+p] (zero-padded)
    cwt = nc.dram_tensor("cwt", [KC, N_KCHUNKS * M_OUT], DT, kind="ExternalInput")

    n_supers = N_PER_CORE // N_SUPER  # 2
    subs = N_SUPER // MM_N            # 8 (one PSUM bank each)
    XSPLIT = 1024                     # columns per x dma_start / tile
    nsplit = N_SUPER // XSPLIT        # 4

    # output in [block, 128, XSPLIT] layout: contiguous per-DMA, 128 partitions
    # (rows 106..127 are padding garbage; host slices them off)
    yt = nc.dram_tensor(
        "yt", [n_supers * nsplit, KC, XSPLIT], F32, kind="ExternalOutput"
    )

    with TileContext(nc) as tc:
        with (
            tc.tile_pool(name="w", bufs=1) as wpool,
            tc.tile_pool(name="x", bufs=6 * nsplit) as xpool,
            tc.tile_pool(name="out", bufs=8) as opool,
            tc.tile_pool(name="psum", bufs=1, space="PSUM") as ppool,
        ):
            wtile = wpool.tile([KC, N_KCHUNKS * M_OUT], DT, tag="w")
            nc.sync.dma_start(out=wtile[:, :], in_=cwt[:, :])

            for sp in range(n_supers):
                c0 = sp * N_SUPER
                psum = ppool.tile([M_OUT, N_SUPER], F32, tag="ps")
                for k in range(N_KCHUNKS):
                    k0 = k * KC
                    kp = min(KC, D_FEAT - k0)
                    xts = []
                    for j in range(nsplit):
                        xtile = xpool.tile([KC, XSPLIT], DT, tag="x")
                        eng = nc.sync if j % 2 == 0 else nc.scalar
                        eng.dma_start(
                            out=xtile[:kp],
                            in_=xt[
                                k0 : k0 + kp,
                                c0 + j * XSPLIT : c0 + (j + 1) * XSPLIT,
                            ],
                        )
                        xts.append(xtile)
                    for s in range(subs):
                        xt_j = xts[(s * MM_N) // XSPLIT]
                        off = (s * MM_N) % XSPLIT
                        nc.tensor.matmul(
                            psum[:, s * MM_N : (s + 1) * MM_N],
                            wtile[:kp, k * M_OUT : (k + 1) * M_OUT],
                            xt_j[:kp, off : off + MM_N],
                            start=(k == 0),
                            stop=(k == N_KCHUNKS - 1),
                        )
                for j in range(nsplit):
                    out_t = opool.tile([KC, XSPLIT], F32, tag="out")
                    nc.vector.tensor_copy(
                        out=out_t[:M_OUT, :],
                        in_=psum[:, j * XSPLIT : (j + 1) * XSPLIT],
                    )
                    nc.sync.dma_start(out=yt[sp * nsplit + j], in_=out_t[:, :])

    nc.compile()
    return nc, 1024


def _build_bf16split():
    import concourse.mybir as mybir
    from concourse import bacc
    from concourse.tile import TileContext

    BF = mybir.dt.bfloat16
    F32 = mybir.dt.float32

    nc = bacc.Bacc("TRN2", target_bir_lowering=False, debug=True)
    xh = nc.dram_tensor("xh", [D_FEAT, N_PER_CORE], BF, kind="ExternalInput")
    xl = nc.dram_tensor("xl", [D_FEAT, N_PER_CORE], BF, kind="ExternalInput")
    cwh = nc.dram_tensor("cwh", [KC, N_KCHUNKS * M_OUT], BF, kind="ExternalInput")
    cwl = nc.dram_tensor("cwl", [KC, N_KCHUNKS * M_OUT], BF, kind="ExternalInput")

    n_super = 2048                    # half PSUM per supergroup -> ping-pong
    n_supers = N_PER_CORE // n_super  # 4
    subs = n_super // MM_N            # 4
    XSPLIT = 2048                     # bf16: 4KB descriptors at 2048 cols
    nsplit = n_super // XSPLIT        # 1

    yt = nc.dram_tensor(
        "yt", [n_supers * nsplit, KC, XSPLIT], F32, kind="ExternalOutput"
    )

    with TileContext(nc) as tc:
        with (
            tc.tile_pool(name="w", bufs=1) as wpool,
            tc.tile_pool(name="x", bufs=7 * 2 * nsplit) as xpool,
            tc.tile_pool(name="out", bufs=4) as opool,
            tc.tile_pool(name="psum", bufs=2, space="PSUM") as ppool,
        ):
            wh = wpool.tile([KC, N_KCHUNKS * M_OUT], BF, tag="wh")
            nc.sync.dma_start(out=wh[:, :], in_=cwh[:, :])
            wl = wpool.tile([KC, N_KCHUNKS * M_OUT], BF, tag="wl")
            nc.scalar.dma_start(out=wl[:, :], in_=cwl[:, :])

            for sp in range(n_supers):
                c0 = sp * n_super
                psum = ppool.tile([M_OUT, n_super], F32, tag="ps")
                for k in range(N_KCHUNKS):
                    k0 = k * KC
                    kp = min(KC, D_FEAT - k0)
                    xh_ts, xl_ts = [], []
                    for j in range(nsplit):
                        cs = slice(c0 + j * XSPLIT, c0 + (j + 1) * XSPLIT)
                        th = xpool.tile([KC, XSPLIT], BF, tag="x")
                        eng = nc.sync if j % 2 == 0 else nc.scalar
                        eng.dma_start(out=th[:kp], in_=xh[k0 : k0 + kp, cs])
                        xh_ts.append(th)
                        tl = xpool.tile([KC, XSPLIT], BF, tag="x")
                        eng = nc.scalar if j % 2 == 0 else nc.sync
                        eng.dma_start(out=tl[:kp], in_=xl[k0 : k0 + kp, cs])
                        xl_ts.append(tl)
                    ws = slice(k * M_OUT, (k + 1) * M_OUT)
                    for s in range(subs):
                        j = (s * MM_N) // XSPLIT
                        off = (s * MM_N) % XSPLIT
                        for wt, xs, st, sp_ in (
                            (wh, xh_ts, k == 0, False),
                            (wl, xh_ts, False, False),
                            (wh, xl_ts, False, k == N_KCHUNKS - 1),
                        ):
                            nc.tensor.matmul(
                                psum[:, s * MM_N : (s + 1) * MM_N],
                                wt[:kp, ws],
                                xs[j][:kp, off : off + MM_N],
                                start=st,
                                stop=sp_,
                            )
                for j in range(nsplit):
                    out_t = opool.tile([KC, XSPLIT], F32, tag="out")
                    nc.vector.tensor_copy(
                        out=out_t[:M_OUT, :],
                        in_=psum[:, j * XSPLIT : (j + 1) * XSPLIT],
                    )
                    eng = nc.sync if (sp * nsplit + j) % 2 == 0 else nc.scalar
                    eng.dma_start(out=yt[sp * nsplit + j], in_=out_t[:, :])

    nc.compile()
    return nc, XSPLIT


def _build_fp16w():
    """Wide variant: 2 supergroups of 4096 cols (1MB / 8KB-row transfers,
    half the DMA-transfer + semaphore count), column-major matmul order with
    per-512-col chunk drains so psum frees incrementally (no pingpong needed
    and near-zero drain tail)."""
    import concourse.mybir as mybir
    from concourse import bacc
    from concourse.tile import TileContext

    F16 = mybir.dt.float16
    F32 = mybir.dt.float32
    I16 = mybir.dt.int16
    MP = 128

    nc = bacc.Bacc("TRN2", target_bir_lowering=False, debug=True)
    xt = nc.dram_tensor("xt", [D_FEAT, N_PER_CORE], F16, kind="ExternalInput")
    cw = nc.dram_tensor("cw", [KC, N_KCHUNKS * MP], F16, kind="ExternalInput")

    n_super = 4096
    n_supers = N_PER_CORE // n_super  # 2
    subs = n_super // MM_N            # 8

    ys = nc.dram_tensor("ys", [n_supers, N_REL, n_super], I16, kind="ExternalOutput")
    yp = nc.dram_tensor("yp", [n_supers, N_REL, n_super], F16, kind="ExternalOutput")

    with TileContext(nc) as tc:
        with (
            tc.tile_pool(name="w", bufs=1) as wpool,
            tc.tile_pool(name="x", bufs=19) as xpool,
            tc.tile_pool(name="out", bufs=2) as opool,
            tc.tile_pool(name="psum", bufs=1, space="PSUM") as ppool,
        ):
            wtile = wpool.tile([KC, N_KCHUNKS * MP], F16, tag="w")
            nc.sync.dma_start(out=wtile[:, :], in_=cw[:, :])

            for sp in range(n_supers):
                c0 = sp * n_super
                psum = ppool.tile([MP, n_super], F32, tag="ps")
                ys_t = opool.tile([N_REL, n_super], I16, tag="ys")
                yp_t = opool.tile([N_REL, n_super], F16, tag="yp")
                xts = []
                for k in range(N_KCHUNKS):
                    k0 = k * KC
                    kp = min(KC, D_FEAT - k0)
                    xtile = xpool.tile([KC, n_super], F16, tag="x")
                    eng = nc.sync if k % 2 == 0 else nc.scalar
                    eng.dma_start(
                        out=xtile[:kp], in_=xt[k0 : k0 + kp, c0 : c0 + n_super]
                    )
                    xts.append((xtile, kp))
                for s in range(subs):
                    cs = slice(s * MM_N, (s + 1) * MM_N)
                    for k in range(N_KCHUNKS):
                        xtile, kp = xts[k]
                        nc.tensor.matmul(
                            psum[:, cs],
                            wtile[:kp, k * MP : (k + 1) * MP],
                            xtile[:kp, cs],
                            start=(k == 0),
                            stop=(k == N_KCHUNKS - 1),
                        )
                    nc.vector.tensor_scalar_mul(ys_t[:, cs], psum[:N_REL, cs], 64.0)
                    nc.vector.tensor_copy(
                        out=yp_t[:, cs], in_=psum[64 : 64 + N_REL, cs]
                    )
                    nc.gpsimd.dma_start(out=ys[sp, :, cs], in_=ys_t[:, cs])
                    nc.gpsimd.dma_start(out=yp[sp, :, cs], in_=yp_t[:, cs])

    nc.compile()
    return nc, n_super


def _build(variant=None):
    variant = variant or VARIANT
    if variant not in _CACHE:
        builders = {
            "fp16": _build_fp16,
            "fp16w": _build_fp16w,
            "bf16split": _build_bf16split,
            "f32r": _build_f32r,
        }
        _CACHE[variant] = builders[variant]()
    return _CACHE[variant]


def _pack_weights(CWT, dtype=np.float32):
    """CWT [D_FEAT, M] -> [128, 17*M] with wpack[p, k*M+m] = CWT[128k+p, m]."""
    M = CWT.shape[1]
    pad = N_KCHUNKS * KC - D_FEAT
    cw = np.concatenate(
        [CWT.astype(np.float32), np.zeros((pad, M), dtype=np.float32)], axis=0
    )
    return np.ascontiguousarray(
        cw.reshape(N_KCHUNKS, KC, M).transpose(1, 0, 2).reshape(KC, -1)
    ).astype(dtype)


def _unpack_yt(res, xsplit, variant=None):
    if (variant or VARIANT) == "fp16":
        S = np.concatenate(
            [
                res.results[c]["ys"]
                .astype(np.float32)
                .transpose(1, 0, 2)
                .reshape(N_REL, N_PER_CORE)
                for c in range(N_CORES)
            ],
            axis=1,
        ) * (1.0 / 64.0)
        P = np.concatenate(
            [
                res.results[c]["yp"]
                .astype(np.float32)
                .transpose(1, 0, 2)
                .reshape(N_REL, N_PER_CORE)
                for c in range(N_CORES)
            ],
            axis=1,
        )
        return np.concatenate([S, P], axis=0)
    return np.concatenate(
        [
            res.results[c]["yt"][:, :M_OUT, :]
            .transpose(1, 0, 2)
            .reshape(M_OUT, N_PER_CORE)
            for c in range(N_CORES)
        ],
        axis=1,
    )


def _ensure_ntff_hook():
    """bass_utils' trace path hard-imports antenv.axon_hooks, which this image
    lacks; shim it so a BASS_TRACE env var (or trace=True) can't crash."""
    import types

    try:
        from antenv.axon_hooks import get_axon_ntff_profile_hook  # noqa: F401

        return
    except ImportError:
        pass
    try:
        import antenv
        from trn_agent_boot.trn_boot import _ntff_profile_via_ctypes

        hook = _ntff_profile_via_ctypes("/opt/axon/libaxon_pjrt.so")
    except Exception:
        antenv, hook = None, None
    mod = types.ModuleType("antenv.axon_hooks")
    _h = [hook]
    mod.set_axon_ntff_profile_hook = lambda h: _h.__setitem__(0, h)
    mod.get_axon_ntff_profile_hook = lambda: _h[0]
    sys.modules["antenv.axon_hooks"] = mod
    if antenv is not None:
        antenv.axon_hooks = mod


def _run_device(XT, CWT, trace=False, variant=None):
    """XT [D_FEAT, N_SENT] f32, CWT [D_FEAT, 106] f32 -> YT [106, N_SENT] f32."""
    _ensure_ntff_hook()
    from concourse.bass_utils import run_bass_kernel_spmd

    variant = variant or VARIANT
    nc, xsplit = _build(variant)

    if variant == "fp16":
        XF = XT.astype(np.float16)
        CWT128 = np.zeros((D_FEAT, 128), dtype=np.float32)
        CWT128[:, :N_REL] = CWT[:, :N_REL]
        CWT128[:, 64 : 64 + N_REL] = CWT[:, N_REL:]
        wpack = _pack_weights(CWT128, np.float16)
        in_maps = [
            {
                "xt": np.ascontiguousarray(
                    XF[:, c * N_PER_CORE : (c + 1) * N_PER_CORE]
                ),
                "cw": wpack,
            }
            for c in range(N_CORES)
        ]
    elif variant == "f32r":
        wpack = _pack_weights(CWT)
        in_maps = [
            {
                "xt": np.ascontiguousarray(
                    XT[:, c * N_PER_CORE : (c + 1) * N_PER_CORE]
                ),
                "cwt": wpack,
            }
            for c in range(N_CORES)
        ]
    else:
        import ml_dtypes

        bf16 = ml_dtypes.bfloat16
        XH = XT.astype(bf16)
        XL = (XT - XH.astype(np.float32)).astype(bf16)
        CWH = CWT.astype(np.float32).astype(bf16).astype(np.float32)
        CWL = CWT.astype(np.float32) - CWH
        wh = _pack_weights(CWH, bf16)
        wl = _pack_weights(CWL, bf16)
        in_maps = [
            {
                "xh": np.ascontiguousarray(
                    XH[:, c * N_PER_CORE : (c + 1) * N_PER_CORE]
                ),
                "xl": np.ascontiguousarray(
                    XL[:, c * N_PER_CORE : (c + 1) * N_PER_CORE]
                ),
                "cwh": wh,
                "cwl": wl,
            }
            for c in range(N_CORES)
        ]

    res = run_bass_kernel_spmd(nc, in_maps, list(range(N_CORES)), trace=trace)
    return _unpack_yt(res, xsplit, variant), res


def kernel(X, Constraints, W, b, X_Scope, X_Rel, _trace=False, _res_out=None):
    X = np.asarray(X)
    Constraints = np.asarray(Constraints)
    W = np.asarray(W)
    b = np.asarray(b)
    X_Scope = np.asarray(X_Scope)
    X_Rel = np.asarray(X_Rel)

    N, D = X.shape
    B = X_Scope.shape[0]
    R = Constraints.shape[0]
    assert (N, D, R) == (N_SENT, D_FEAT, N_REL), (N, D, R)

    XT = np.ascontiguousarray(X.T)
    CWT = np.ascontiguousarray(
        np.concatenate([Constraints, W], axis=0).T.astype(np.float32)
    )

    YT, res = _run_device(XT, CWT, trace=_trace)
    if _res_out is not None:
        _res_out.append(res)

    S_all = YT[:N_REL]          # [53, N] scores for every relation
    P = YT[N_REL:]              # [53, N] per-sentence classifier projections

    # host downstream on [N, 53]-sized data (mirrors reference semantics)
    starts = X_Scope[:, 0].astype(np.int64)
    seg = np.searchsorted(starts, np.arange(N, dtype=np.int64), side="right") - 1
    rel = np.asarray(X_Rel)[seg]  # wraps for seg == -1, same as jnp
    s = S_all[rel, np.arange(N)].astype(np.float64)

    valid = seg >= 0
    segv = seg[valid]
    m = np.full(B, -np.inf)
    np.maximum.at(m, segv, s[valid])
    e = np.exp(s - np.where(valid, m[np.clip(seg, 0, B - 1)], np.inf))
    e = np.where(valid, e, 0.0)
    z = np.bincount(segv, weights=e[valid], minlength=B)
    zsafe = np.where(z == 0.0, 1.0, z)
    w = e / zsafe[np.clip(seg, 0, B - 1)]

    out = np.empty((B, N_REL), dtype=np.float64)
    Pw = P.astype(np.float64) * w[None, :]
    for j in range(N_REL):
        out[:, j] = np.bincount(segv, weights=Pw[j, valid], minlength=B)
    out += b.astype(np.float64)[None, :]
    return out.astype(np.float32)



# revision 23
# speedup vs baseline: 1.0727x; 1.0727x over previous
"""Bass/Trainium2 kernel for nn_CGRE_68719477510 (ragged_sequence).

Restructure: scores[i] = X[i] . Constraints[rel(bag(i))] and the classifier
out = bag @ W.T are both projections of X onto small [53, 2070] matrices.
So one device pass computes Y = [Constraints; W] @ X.T  ([106, N]) — the only
traffic proportional to X. The segment softmax + weighted sum then operate
on the projected [N, 53] rows (P = X @ W.T), never touching X again:
    out[bag] = sum_i softmax_i(S) * P[i]  ==  (sum_i w_i X_i) @ W.T
Sharding: split sentences N=65536 into 8 contiguous chunks of 8192 (one per
core); replicate the small combined weight. The ragged segment ops run on
host over the tiny [N, 53] projection.

Precision: X and weights ship as fp16 (halves HBM traffic vs f32-equivalent
pairs; ~1.4e-3 output Frobenius error vs the 2e-2 gate). The stationary is
padded to 128 with C rows at partitions 0-52 and W rows at 64-116 so both
PSUM reads are quadrant-aligned. Scores leave the device as int16 x64
fixed point (|s| < 240, abs err 2^-7), P as fp16 — halving output bytes.

Performance notes (from perfetto traces): reads spray across all 16 DMA
engines, but hwdge (sync/scalar) WRITES share the 8 DMAHW hardware queues
with the load stream and break their +16-per-transfer semaphore accounting,
stalling loads ~15us per supergroup boundary — so all output writes ride
gpsimd's software DGE queues instead. 231us (bf16 hi/lo baseline) -> 131us.
"""

import sys

sys.path.insert(0, "/opt/trn_rl_repo")

import numpy as np

N_SENT = 65536
D_FEAT = 2070
N_REL = 53
N_CORES = 8
N_PER_CORE = N_SENT // N_CORES  # 8192
M_OUT = 2 * N_REL  # 106 rows: [Constraints; W]

KC = 128                      # contraction chunk (partition dim)
N_SUPER = 4096                # sentences per supergroup (fills all 8 PSUM banks)
MM_N = 512                    # moving free dim per matmul (one PSUM bank)
N_KCHUNKS = (D_FEAT + KC - 1) // KC  # 17 (16x128 + 22)

VARIANT = "fp16w"             # "fp16w" | "fp16" | "bf16split" | "f32r"

_CACHE = {}


def _build_fp16():
    """Single-pass fp16 variant: X and [Constraints; W] both fp16.

    Halves HBM traffic vs the bf16 hi/lo pair (33.9 MB/core vs 67.9) and
    cuts PE work 3x (one matmul per tile). fp16's 10-bit mantissa keeps
    score error ~0.01 abs -> ~0.14% output Frobenius error (measured on
    the real inputs), 15x under the 2e-2 gate.
    """
    import concourse.mybir as mybir
    from concourse import bacc
    from concourse.tile import TileContext

    F16 = mybir.dt.float16
    F32 = mybir.dt.float32
    I16 = mybir.dt.int16
    MP = 128  # stationary padded to 128: C rows at 0..52, W rows at 64..116
              # (PSUM partition reads must start quadrant-aligned: 0 / 64)

    nc = bacc.Bacc("TRN2", target_bir_lowering=False, debug=True)
    xt = nc.dram_tensor("xt", [D_FEAT, N_PER_CORE], F16, kind="ExternalInput")
    cw = nc.dram_tensor("cw", [KC, N_KCHUNKS * MP], F16, kind="ExternalInput")

    n_super = 2048                    # 4 PSUM banks -> ping-pong 2 tiles
    n_supers = N_PER_CORE // n_super  # 4
    subs = n_super // MM_N            # 4

    # DMA writes pin to one engine per queue (sync->0, scalar->1, gpsimd->
    # {6,7}) instead of spraying like strided reads, so output bytes are the
    # scarce resource: scores ship as int16 (x64 fixed point, |s|<240 so no
    # overflow) and P as fp16, balanced across all four write sinks.
    ys = nc.dram_tensor("ys", [n_supers, N_REL, n_super], I16, kind="ExternalOutput")
    yp = nc.dram_tensor("yp", [n_supers, N_REL, n_super], F16, kind="ExternalOutput")

    with TileContext(nc) as tc:
        with (
            tc.tile_pool(name="w", bufs=1) as wpool,
            tc.tile_pool(name="x", bufs=20) as xpool,
            tc.tile_pool(name="out", bufs=4) as opool,
            tc.tile_pool(name="psum", bufs=2, space="PSUM") as ppool,
        ):
            wtile = wpool.tile([KC, N_KCHUNKS * MP], F16, tag="w")
            nc.sync.dma_start(out=wtile[:, :], in_=cw[:, :])

            # ALL output writes go through gpsimd's software DGE queues.
            # x loads and hwdge writes share the 8 DMAHW hardware queues and
            # their +16-per-transfer semaphore accounting; single-engine-
            # pinned writes break the expected increment pattern and stall
            # the load stream ~15us at every supergroup boundary. The swdge
            # (DMASW) queues are accounted separately, so writes there never
            # perturb the load pipeline (engines 6/7 absorb ~17us of write
            # descriptors, still below the 83us x floor).
            for sp in range(n_supers):
                c0 = sp * n_super
                psum = ppool.tile([MP, n_super], F32, tag="ps")
                for k in range(N_KCHUNKS):
                    k0 = k * KC
                    kp = min(KC, D_FEAT - k0)
                    xtile = xpool.tile([KC, n_super], F16, tag="x")
                    eng = nc.sync if k % 2 == 0 else nc.scalar
                    eng.dma_start(
                        out=xtile[:kp], in_=xt[k0 : k0 + kp, c0 : c0 + n_super]
                    )
                    ws = slice(k * MP, (k + 1) * MP)
                    for s in range(subs):
                        nc.tensor.matmul(
                            psum[:, s * MM_N : (s + 1) * MM_N],
                            wtile[:kp, ws],
                            xtile[:kp, s * MM_N : (s + 1) * MM_N],
                            start=(k == 0),
                            stop=(k == N_KCHUNKS - 1),
                        )
                # drain in column chunks: the last super pipelines its casts
                # and writes behind the trailing matmuls, shrinking the tail
                nchunk = 4 if sp == n_supers - 1 else 1
                cw_ = n_super // nchunk
                ys_t = opool.tile([N_REL, n_super], I16, tag="ys")
                yp_t = opool.tile([N_REL, n_super], F16, tag="yp")
                for j in range(nchunk):
                    cs = slice(j * cw_, (j + 1) * cw_)
                    nc.vector.tensor_scalar_mul(ys_t[:, cs], psum[:N_REL, cs], 64.0)
                    nc.vector.tensor_copy(
                        out=yp_t[:, cs], in_=psum[64 : 64 + N_REL, cs]
                    )
                    nc.gpsimd.dma_start(out=ys[sp, :, cs], in_=ys_t[:, cs])
                    nc.gpsimd.dma_start(out=yp[sp, :, cs], in_=yp_t[:, cs])

    nc.compile()
    return nc, n_super


def _build_fp16w():
    """Wide fp16 variant: 2 supergroups of 4096 cols -> 34 x-transfers of
    1MB (8KB rows) instead of 68 x 512KB, halving DMA-transfer + semaphore
    overhead and supergroup boundaries. Single [128, 4096] psum (all 8
    banks, no ping-pong): the ~9us drain bubble hits only the PE while the
    deep x pool keeps the DMA engines streaming."""
    import concourse.mybir as mybir
    from concourse import bacc
    from concourse.tile import TileContext

    F16 = mybir.dt.float16
    F32 = mybir.dt.float32
    I16 = mybir.dt.int16
    MP = 128

    nc = bacc.Bacc("TRN2", target_bir_lowering=False, debug=True)
    xt = nc.dram_tensor("xt", [D_FEAT, N_PER_CORE], F16, kind="ExternalInput")
    cw = nc.dram_tensor("cw", [KC, N_KCHUNKS * MP], F16, kind="ExternalInput")

    n_super = 4096
    n_supers = N_PER_CORE // n_super  # 2
    subs = n_super // MM_N            # 8

    ys = nc.dram_tensor("ys", [n_supers, N_REL, n_super], I16, kind="ExternalOutput")
    yp = nc.dram_tensor("yp", [n_supers, N_REL, n_super], F16, kind="ExternalOutput")

    with TileContext(nc) as tc:
        with (
            tc.tile_pool(name="w", bufs=1) as wpool,
            tc.tile_pool(name="x", bufs=12) as xpool,
            tc.tile_pool(name="out", bufs=2) as opool,
            tc.tile_pool(name="psum", bufs=1, space="PSUM") as ppool,
        ):
            wtile = wpool.tile([KC, N_KCHUNKS * MP], F16, tag="w")
            nc.sync.dma_start(out=wtile[:, :], in_=cw[:, :])

            for sp in range(n_supers):
                c0 = sp * n_super
                psum = ppool.tile([MP, n_super], F32, tag="ps")
                for k in range(N_KCHUNKS):
                    k0 = k * KC
                    kp = min(KC, D_FEAT - k0)
                    xtile = xpool.tile([KC, n_super], F16, tag="x")
                    eng = nc.sync if k % 2 == 0 else nc.scalar
                    eng.dma_start(
                        out=xtile[:kp], in_=xt[k0 : k0 + kp, c0 : c0 + n_super]
                    )
                    ws = slice(k * MP, (k + 1) * MP)
                    for s in range(subs):
                        nc.tensor.matmul(
                            psum[:, s * MM_N : (s + 1) * MM_N],
                            wtile[:kp, ws],
                            xtile[:kp, s * MM_N : (s + 1) * MM_N],
                            start=(k == 0),
                            stop=(k == N_KCHUNKS - 1),
                        )
                nchunk = 4 if sp == n_supers - 1 else 2
                cw_ = n_super // nchunk
                ys_t = opool.tile([N_REL, n_super], I16, tag="ys")
                yp_t = opool.tile([N_REL, n_super], F16, tag="yp")
                for j in range(nchunk):
                    cs = slice(j * cw_, (j + 1) * cw_)
                    nc.vector.tensor_scalar_mul(ys_t[:, cs], psum[:N_REL, cs], 64.0)
                    nc.vector.tensor_copy(
                        out=yp_t[:, cs], in_=psum[64 : 64 + N_REL, cs]
                    )
                    nc.gpsimd.dma_start(out=ys[sp, :, cs], in_=ys_t[:, cs])
                    nc.gpsimd.dma_start(out=yp[sp, :, cs], in_=yp_t[:, cs])

    nc.compile()
    return nc, n_super


def _build_f32r():
    import concourse.mybir as mybir
    from concourse import bacc
    from concourse.tile import TileContext

    DT = mybir.dt.float32r  # fp32 bits, full-rate PE streaming mode
    F32 = mybir.dt.float32

    nc = bacc.Bacc("TRN2", target_bir_lowering=False, debug=True)
    xt = nc.dram_tensor("xt", [D_FEAT, N_PER_CORE], DT, kind="ExternalInput")
    # weights packed on host: wpack[p, k*106+m] = CW[m, 128k+p] (zero-padded)
    cwt = nc.dram_tensor("cwt", [KC, N_KCHUNKS * M_OUT], DT, kind="ExternalInput")

    n_supers = N_PER_CORE // N_SUPER  # 2
    subs = N_SUPER // MM_N            # 8 (one PSUM bank each)
    XSPLIT = 1024                     # columns per x dma_start / tile
    nsplit = N_SUPER // XSPLIT        # 4

    # output in [block, 128, XSPLIT] layout: contiguous per-DMA, 128 partitions
    # (rows 106..127 are padding garbage; host slices them off)
    yt = nc.dram_tensor(
        "yt", [n_supers * nsplit, KC, XSPLIT], F32, kind="ExternalOutput"
    )

    with TileContext(nc) as tc:
        with (
            tc.tile_pool(name="w", bufs=1) as wpool,
            tc.tile_pool(name="x", bufs=6 * nsplit) as xpool,
            tc.tile_pool(name="out", bufs=8) as opool,
            tc.tile_pool(name="psum", bufs=1, space="PSUM") as ppool,
        ):
            wtile = wpool.tile([KC, N_KCHUNKS * M_OUT], DT, tag="w")
            nc.sync.dma_start(out=wtile[:, :], in_=cwt[:, :])

            for sp in range(n_supers):
                c0 = sp * N_SUPER
                psum = ppool.tile([M_OUT, N_SUPER], F32, tag="ps")
                for k in range(N_KCHUNKS):
                    k0 = k * KC
                    kp = min(KC, D_FEAT - k0)
                    xts = []
                    for j in range(nsplit):
                        xtile = xpool.tile([KC, XSPLIT], DT, tag="x")
                        eng = nc.sync if j % 2 == 0 else nc.scalar
                        eng.dma_start(
                            out=xtile[:kp],
                            in_=xt[
                                k0 : k0 + kp,
                                c0 + j * XSPLIT : c0 + (j + 1) * XSPLIT,
                            ],
                        )
                        xts.append(xtile)
                    for s in range(subs):
                        xt_j = xts[(s * MM_N) // XSPLIT]
                        off = (s * MM_N) % XSPLIT
                        nc.tensor.matmul(
                            psum[:, s * MM_N : (s + 1) * MM_N],
                            wtile[:kp, k * M_OUT : (k + 1) * M_OUT],
                            xt_j[:kp, off : off + MM_N],
                            start=(k == 0),
                            stop=(k == N_KCHUNKS - 1),
                        )
                for j in range(nsplit):
                    out_t = opool.tile([KC, XSPLIT], F32, tag="out")
                    nc.vector.tensor_copy(
                        out=out_t[:M_OUT, :],
                        in_=psum[:, j * XSPLIT : (j + 1) * XSPLIT],
                    )
                    nc.sync.dma_start(out=yt[sp * nsplit + j], in_=out_t[:, :])

    nc.compile()
    return nc, 1024


def _build_bf16split():
    import concourse.mybir as mybir
    from concourse import bacc
    from concourse.tile import TileContext

    BF = mybir.dt.bfloat16
    F32 = mybir.dt.float32

    nc = bacc.Bacc("TRN2", target_bir_lowering=False, debug=True)
    xh = nc.dram_tensor("xh", [D_FEAT, N_PER_CORE], BF, kind="ExternalInput")
    xl = nc.dram_tensor("xl", [D_FEAT, N_PER_CORE], BF, kind="ExternalInput")
    cwh = nc.dram_tensor("cwh", [KC, N_KCHUNKS * M_OUT], BF, kind="ExternalInput")
    cwl = nc.dram_tensor("cwl", [KC, N_KCHUNKS * M_OUT], BF, kind="ExternalInput")

    n_super = 2048                    # half PSUM per supergroup -> ping-pong
    n_supers = N_PER_CORE // n_super  # 4
    subs = n_super // MM_N            # 4
    XSPLIT = 2048                     # bf16: 4KB descriptors at 2048 cols
    nsplit = n_super // XSPLIT        # 1

    yt = nc.dram_tensor(
        "yt", [n_supers * nsplit, KC, XSPLIT], F32, kind="ExternalOutput"
    )

    with TileContext(nc) as tc:
        with (
            tc.tile_pool(name="w", bufs=1) as wpool,
            tc.tile_pool(name="x", bufs=7 * 2 * nsplit) as xpool,
            tc.tile_pool(name="out", bufs=4) as opool,
            tc.tile_pool(name="psum", bufs=2, space="PSUM") as ppool,
        ):
            wh = wpool.tile([KC, N_KCHUNKS * M_OUT], BF, tag="wh")
            nc.sync.dma_start(out=wh[:, :], in_=cwh[:, :])
            wl = wpool.tile([KC, N_KCHUNKS * M_OUT], BF, tag="wl")
            nc.scalar.dma_start(out=wl[:, :], in_=cwl[:, :])

            for sp in range(n_supers):
                c0 = sp * n_super
                psum = ppool.tile([M_OUT, n_super], F32, tag="ps")
                for k in range(N_KCHUNKS):
                    k0 = k * KC
                    kp = min(KC, D_FEAT - k0)
                    xh_ts, xl_ts = [], []
                    for j in range(nsplit):
                        cs = slice(c0 + j * XSPLIT, c0 + (j + 1) * XSPLIT)
                        th = xpool.tile([KC, XSPLIT], BF, tag="x")
                        eng = nc.sync if j % 2 == 0 else nc.scalar
                        eng.dma_start(out=th[:kp], in_=xh[k0 : k0 + kp, cs])
                        xh_ts.append(th)
                        tl = xpool.tile([KC, XSPLIT], BF, tag="x")
                        eng = nc.scalar if j % 2 == 0 else nc.sync
                        eng.dma_start(out=tl[:kp], in_=xl[k0 : k0 + kp, cs])
                        xl_ts.append(tl)
                    ws = slice(k * M_OUT, (k + 1) * M_OUT)
                    for s in range(subs):
                        j = (s * MM_N) // XSPLIT
                        off = (s * MM_N) % XSPLIT
                        for wt, xs, st, sp_ in (
                            (wh, xh_ts, k == 0, False),
                            (wl, xh_ts, False, False),
                            (wh, xl_ts, False, k == N_KCHUNKS - 1),
                        ):
                            nc.tensor.matmul(
                                psum[:, s * MM_N : (s + 1) * MM_N],
                                wt[:kp, ws],
                                xs[j][:kp, off : off + MM_N],
                                start=st,
                                stop=sp_,
                            )
                for j in range(nsplit):
                    out_t = opool.tile([KC, XSPLIT], F32, tag="out")
                    nc.vector.tensor_copy(
                        out=out_t[:M_OUT, :],
                        in_=psum[:, j * XSPLIT : (j + 1) * XSPLIT],
                    )
                    eng = nc.sync if (sp * nsplit + j) % 2 == 0 else nc.scalar
                    eng.dma_start(out=yt[sp * nsplit + j], in_=out_t[:, :])

    nc.compile()
    return nc, XSPLIT


def _build(variant=None):
    variant = variant or VARIANT
    if variant not in _CACHE:
        builders = {
            "fp16": _build_fp16,
            "fp16w": _build_fp16w,
            "bf16split": _build_bf16split,
            "f32r": _build_f32r,
        }
        _CACHE[variant] = builders[variant]()
    return _CACHE[variant]


def _pack_weights(CWT, dtype=np.float32):
    """CWT [D_FEAT, M] -> [128, 17*M] with wpack[p, k*M+m] = CWT[128k+p, m]."""
    M = CWT.shape[1]
    pad = N_KCHUNKS * KC - D_FEAT
    cw = np.concatenate(
        [CWT.astype(np.float32), np.zeros((pad, M), dtype=np.float32)], axis=0
    )
    return np.ascontiguousarray(
        cw.reshape(N_KCHUNKS, KC, M).transpose(1, 0, 2).reshape(KC, -1)
    ).astype(dtype)


def _unpack_yt(res, xsplit, variant=None):
    if (variant or VARIANT) in ("fp16", "fp16w"):
        S = np.concatenate(
            [
                res.results[c]["ys"]
                .astype(np.float32)
                .transpose(1, 0, 2)
                .reshape(N_REL, N_PER_CORE)
                for c in range(N_CORES)
            ],
            axis=1,
        ) * (1.0 / 64.0)
        P = np.concatenate(
            [
                res.results[c]["yp"]
                .astype(np.float32)
                .transpose(1, 0, 2)
                .reshape(N_REL, N_PER_CORE)
                for c in range(N_CORES)
            ],
            axis=1,
        )
        return np.concatenate([S, P], axis=0)
    return np.concatenate(
        [
            res.results[c]["yt"][:, :M_OUT, :]
            .transpose(1, 0, 2)
            .reshape(M_OUT, N_PER_CORE)
            for c in range(N_CORES)
        ],
        axis=1,
    )


def _ensure_ntff_hook():
    """bass_utils' trace path hard-imports antenv.axon_hooks, which this image
    lacks; shim it so a BASS_TRACE env var (or trace=True) can't crash."""
    import types

    try:
        from antenv.axon_hooks import get_axon_ntff_profile_hook  # noqa: F401

        return
    except ImportError:
        pass
    try:
        import antenv
        from trn_agent_boot.trn_boot import _ntff_profile_via_ctypes

        hook = _ntff_profile_via_ctypes("/opt/axon/libaxon_pjrt.so")
    except Exception:
        antenv, hook = None, None
    mod = types.ModuleType("antenv.axon_hooks")
    _h = [hook]
    mod.set_axon_ntff_profile_hook = lambda h: _h.__setitem__(0, h)
    mod.get_axon_ntff_profile_hook = lambda: _h[0]
    sys.modules["antenv.axon_hooks"] = mod
    if antenv is not None:
        antenv.axon_hooks = mod


def _run_device(XT, CWT, trace=False, variant=None):
    """XT [D_FEAT, N_SENT] f32, CWT [D_FEAT, 106] f32 -> YT [106, N_SENT] f32."""
    _ensure_ntff_hook()
    from concourse.bass_utils import run_bass_kernel_spmd

    variant = variant or VARIANT
    nc, xsplit = _build(variant)

    if variant in ("fp16", "fp16w"):
        XF = XT.astype(np.float16)
        CWT128 = np.zeros((D_FEAT, 128), dtype=np.float32)
        CWT128[:, :N_REL] = CWT[:, :N_REL]
        CWT128[:, 64 : 64 + N_REL] = CWT[:, N_REL:]
        wpack = _pack_weights(CWT128, np.float16)
        in_maps = [
            {
                "xt": np.ascontiguousarray(
                    XF[:, c * N_PER_CORE : (c + 1) * N_PER_CORE]
                ),
                "cw": wpack,
            }
            for c in range(N_CORES)
        ]
    elif variant == "f32r":
        wpack = _pack_weights(CWT)
        in_maps = [
            {
                "xt": np.ascontiguousarray(
                    XT[:, c * N_PER_CORE : (c + 1) * N_PER_CORE]
                ),
                "cwt": wpack,
            }
            for c in range(N_CORES)
        ]
    else:
        import ml_dtypes

        bf16 = ml_dtypes.bfloat16
        XH = XT.astype(bf16)
        XL = (XT - XH.astype(np.float32)).astype(bf16)
        CWH = CWT.astype(np.float32).astype(bf16).astype(np.float32)
        CWL = CWT.astype(np.float32) - CWH
        wh = _pack_weights(CWH, bf16)
        wl = _pack_weights(CWL, bf16)
        in_maps = [
            {
                "xh": np.ascontiguousarray(
                    XH[:, c * N_PER_CORE : (c + 1) * N_PER_CORE]
                ),
                "xl": np.ascontiguousarray(
                    XL[:, c * N_PER_CORE : (c + 1) * N_PER_CORE]
                ),
                "cwh": wh,
                "cwl": wl,
            }
            for c in range(N_CORES)
        ]

    res = run_bass_kernel_spmd(nc, in_maps, list(range(N_CORES)), trace=trace)
    return _unpack_yt(res, xsplit, variant), res


def kernel(X, Constraints, W, b, X_Scope, X_Rel, _trace=False, _res_out=None):
    X = np.asarray(X)
    Constraints = np.asarray(Constraints)
    W = np.asarray(W)
    b = np.asarray(b)
    X_Scope = np.asarray(X_Scope)
    X_Rel = np.asarray(X_Rel)

    N, D = X.shape
    B = X_Scope.shape[0]
    R = Constraints.shape[0]
    assert (N, D, R) == (N_SENT, D_FEAT, N_REL), (N, D, R)

    XT = np.ascontiguousarray(X.T)
    CWT = np.ascontiguousarray(
        np.concatenate([Constraints, W], axis=0).T.astype(np.float32)
    )

    YT, res = _run_device(XT, CWT, trace=_trace)
    if _res_out is not None:
        _res_out.append(res)

    S_all = YT[:N_REL]          # [53, N] scores for every relation
    P = YT[N_REL:]              # [53, N] per-sentence classifier projections

    # host downstream on [N, 53]-sized data (mirrors reference semantics)
    starts = X_Scope[:, 0].astype(np.int64)
    seg = np.searchsorted(starts, np.arange(N, dtype=np.int64), side="right") - 1
    rel = np.asarray(X_Rel)[seg]  # wraps for seg == -1, same as jnp
    s = S_all[rel, np.arange(N)].astype(np.float64)

    valid = seg >= 0
    segv = seg[valid]
    m = np.full(B, -np.inf)
    np.maximum.at(m, segv, s[valid])
    e = np.exp(s - np.where(valid, m[np.clip(seg, 0, B - 1)], np.inf))
    e = np.where(valid, e, 0.0)
    z = np.bincount(segv, weights=e[valid], minlength=B)
    zsafe = np.where(z == 0.0, 1.0, z)
    w = e / zsafe[np.clip(seg, 0, B - 1)]

    out = np.empty((B, N_REL), dtype=np.float64)
    Pw = P.astype(np.float64) * w[None, :]
    for j in range(N_REL):
        out[:, j] = np.bincount(segv, weights=Pw[j, valid], minlength=B)
    out += b.astype(np.float64)[None, :]
    return out.astype(np.float32)

